# revision 1
# baseline (speedup 1.0000x reference)
"""Masked multi-head attention on 8 NeuronCores (faithful torch raw-view semantics).

The reference reshapes (bs, sql, nh*edim) -> (bs, nh, sql, edim) as a RAW VIEW:
head h's length-1024 pseudo-sequence is built from x rows 128h..128h+127 (each
row contributes 8 pseudo-positions, one per 256-col block of the projection),
and output rows 128h..128h+128 depend only on head h. So the work splits into
32 independent (batch, head) pairs -> 4 per core, no cross-core reduction.

Per (b, h): Q/K/V = x[b,128h:128h+128] @ w{q,k,v}.T + b (full 2048-wide), viewed
as (1024, 256) row-major. We index pseudo-positions in PERMUTED order
u = cb*128 + r (s' = r*8 + cb), consistently on both attention axes, which makes
every layout a contiguous matmul tile. The causal mask (on s') is precomputed
host-side for this ordering (16 bf16 tiles, shared by all heads/cores).

Transposed-score formulation: S^T = K^T.T @ Q^T with d-major Q^T/K^T straight
from the projection matmuls; softmax denominator = ones-column matmul; P^T feeds
P@V directly. No on-chip transposes anywhere. Q weights/bias pre-scaled by 1/16.
"""

import sys

sys.path.insert(0, "/opt/trn_rl_repo")

import ml_dtypes
import numpy as np

from concourse import bacc, mybir
from concourse.tile import TileContext
from concourse.bass_utils import run_bass_kernel_spmd

EDIM = 256
BS = 4
SQL = 1024
HPC = 4           # heads per core
NCORES = 8
FDT = mybir.dt.float32
RDT = mybir.dt.float32r    # matmul-feeding storage: 4x faster PE, rounded fp32
NEG = -1.0e30

_cache = {}


def _build():
    nc = bacc.Bacc(dynamic_dma_scratch_size=512)

    xt0 = nc.declare_dram_parameter("xt0", [128, 512], RDT, isOutput=False)
    xt1 = nc.declare_dram_parameter("xt1", [128, 512], RDT, isOutput=False)
    wqk0 = nc.declare_dram_parameter("wqk0", [128, 4096], RDT, isOutput=False)
    wqk1 = nc.declare_dram_parameter("wqk1", [128, 4096], RDT, isOutput=False)
    wv0 = nc.declare_dram_parameter("wv0", [128, 2048], RDT, isOutput=False)
    wv1 = nc.declare_dram_parameter("wv1", [128, 2048], RDT, isOutput=False)
    bqk = nc.declare_dram_parameter("bqk", [128, 32], FDT, isOutput=False)
    bv = nc.declare_dram_parameter("bv", [1, 2048], RDT, isOutput=False)
    mask = nc.declare_dram_parameter("mask", [16, 128, 512], mybir.dt.bfloat16,
                                     isOutput=False)
    wot = nc.declare_dram_parameter("wot", [128, 4096], RDT, isOutput=False)
    onr = nc.declare_dram_parameter("onr", [1, 128], RDT, isOutput=False)
    idn = nc.declare_dram_parameter("idn", [128, 128], mybir.dt.bfloat16,
                                    isOutput=False)
    onc = nc.declare_dram_parameter("onc", [128, 128], RDT, isOutput=False)
    y = nc.declare_dram_parameter("y", [512, 256], FDT, isOutput=True)

    with TileContext(nc) as tc:
        with (
            tc.tile_pool(name="const", bufs=1) as cpool,
            tc.tile_pool(name="w4k", bufs=3) as wqpool,
            tc.tile_pool(name="v2k", bufs=6) as vpool,
            tc.tile_pool(name="qk4k", bufs=4) as qkpool,
            tc.tile_pool(name="work", bufs=2) as wpool,
            tc.tile_pool(name="ps_a", bufs=4, space="PSUM") as ps_a,
            tc.tile_pool(name="ps_o", bufs=1, space="PSUM") as ps_o,
            tc.tile_pool(name="ps_se", bufs=1, space="PSUM") as ps_se,
        ):
            def load(pool, name, src, shape, dt=FDT, tag=None):
                t = pool.tile(shape, dt, tag=tag or name, name=name)
                nc.sync.dma_start(out=t[:, :], in_=src)
                return t

            def mm(out, lhsT, rhs, **kw):
                nc.tensor.matmul(out, lhsT, rhs, **kw)

            xt_sb = [load(cpool, "xt0", xt0[:, :], [128, 512], dt=RDT),
                     load(cpool, "xt1", xt1[:, :], [128, 512], dt=RDT)]
            bqk_sb = load(cpool, "bqk", bqk[:, :], [128, 32])
            wqk_srcs = [wqk0, wqk1]
            wqk_sb = []
            for d in range(2):
                t = wqpool.tile([128, 4096], RDT, tag="w4k", name=f"wqk{d}")
                for ch in range(4):
                    nc.sync.dma_start(
                        out=t[:, ch * 1024:(ch + 1) * 1024],
                        in_=wqk_srcs[d][:, ch * 1024:(ch + 1) * 1024],
                    )
                wqk_sb.append(t)
            wv_sb = [load(vpool, "wv0", wv0[:, :], [128, 2048], dt=RDT, tag="v2k"),
                     load(vpool, "wv1", wv1[:, :], [128, 2048], dt=RDT, tag="v2k")]
            bv_sb = load(cpool, "bv", bv[:, :], [1, 2048], dt=RDT)

            ones_r = load(cpool, "ones_r", onr[:, :], [1, 128], dt=RDT)
            idn_sb = load(cpool, "idn", idn[:, :], [128, 128],
                          dt=mybir.dt.bfloat16)
            ones_c = load(cpool, "ones_c", onc[:, :], [128, 128], dt=RDT)

            # d-major Q^T/K^T, packed per d-chunk: cols = hl*1024 + u
            qt = [qkpool.tile([128, 4096], RDT, tag="qk4k", name=f"qt{c}")
                  for c in range(2)]
            kt = [qkpool.tile([128, 4096], RDT, tag="qk4k", name=f"kt{c}")
                  for c in range(2)]
            # V per head, u-major: [128 r, cb*256 + d]
            v_sb = [vpool.tile([128, 2048], RDT, tag="v2k", name=f"v{i}")
                    for i in range(HPC)]

            # ---- P1: Q^T/K^T projections, d-major (all 4 heads at once) ----
            for s in range(2):          # 0 = Q, 1 = K
                dst = qt if s == 0 else kt
                for cb in range(8):
                    for c in range(2):
                        ps = ps_a.tile([128, 512], FDT, tag="proj", name="proj")
                        for d in range(2):
                            mm(
                                ps[:, :],
                                wqk_sb[d][:, s * 2048 + cb * 256 + c * 128:
                                          s * 2048 + cb * 256 + c * 128 + 128],
                                xt_sb[d][:, :],
                                start=(d == 0), stop=(d == 1),
                            )
                        bi = s * 16 + cb * 2 + c
                        nc.vector.tensor_scalar_add(
                            out=dst[c].rearrange("p (h u) -> p h u", h=4)
                                [:, :, cb * 128:(cb + 1) * 128],
                            in0=ps.rearrange("p (h r) -> p h r", h=4)[:, :, :],
                            scalar1=bqk_sb[:, bi:bi + 1],
                        )

            # ---- P2: V projection per head (t-major == u-major here) ----
            for hl in range(HPC):
                for nn in range(4):
                    ps = ps_a.tile([128, 512], FDT, tag="proj", name="vproj")
                    for d in range(2):
                        mm(
                            ps[:, :],
                            xt_sb[d][:, hl * 128:hl * 128 + 128],
                            wv_sb[d][:, nn * 512:(nn + 1) * 512],
                            start=(d == 0), stop=False,
                        )
                    mm(
                        ps[:, :],
                        ones_r[:, :],
                        bv_sb[:, nn * 512:(nn + 1) * 512],
                        start=False, stop=True,
                    )
                    nc.vector.tensor_copy(
                        out=v_sb[hl][:, nn * 512:(nn + 1) * 512], in_=ps[:, :]
                    )

            # output of attention, normalized, packed: cols = hl*1024 + u
            oh = [wqpool.tile([128, 4096], RDT, tag="w4k", name=f"oh{c}")
                  for c in range(2)]
            wot_sb = load(wqpool, "wot", wot[:, :], [128, 4096], dt=RDT, tag="w4k")

            mask_sb = [load(cpool, f"mask{i}", mask[i], [128, 512],
                            mybir.dt.bfloat16) for i in range(16)]

            # ---- P3: attention per head ----
            for hl in range(HPC):
                for qj in range(2):
                    po = [ps_o.tile([128, 512], FDT, tag="po", name=f"po{c}",
                                    bufs=3) for c in range(2)]
                    se = ps_se.tile([128, 512], FDT, tag="se", name="se",
                                    bufs=1)
                    for ki in range(8):
                        sp = ps_a.tile([128, 512], FDT, tag="proj",
                                       name="score")
                        for c in range(2):
                            mm(
                                sp[:, :],
                                kt[c][:, hl * 1024 + ki * 128:
                                      hl * 1024 + ki * 128 + 128],
                                qt[c][:, hl * 1024 + qj * 512:
                                      hl * 1024 + qj * 512 + 512],
                                start=(c == 0), stop=(c == 1),
                            )
                        pt = wpool.tile([128, 512], RDT, tag="pt", name="pt",
                                        bufs=6)
                        nc.vector.tensor_add(
                            out=pt[:, :], in0=sp[:, :],
                            in1=mask_sb[ki * 2 + qj][:, :],
                        )
                        nc.scalar.activation(
                            pt[:, :], pt[:, :],
                            mybir.ActivationFunctionType.Exp
                        )
                        for c in range(2):
                            mm(
                                po[c][:, :],
                                v_sb[hl][:, ki * 256 + c * 128:
                                         ki * 256 + c * 128 + 128],
                                pt[:, :],
                                start=(ki == 0), stop=(ki == 7),
                            )
                        mm(
                            se[:, :], ones_c[:, :], pt[:, :],
                            start=(ki == 0), stop=(ki == 7),
                        )
                    rc = wpool.tile([128, 512], FDT, tag="rc", name="rc",
                                    bufs=3)
                    nc.vector.reciprocal(out=rc[:, :], in_=se[:, :])
                    for c in range(2):
                        nc.vector.tensor_mul(
                            out=oh[c][:, hl * 1024 + qj * 512:
                                      hl * 1024 + qj * 512 + 512],
                            in0=po[c][:, :], in1=rc[:, :],
                        )

            # ---- P4: output projection per head (y rows are per-head!) ----
            for hl in range(HPC):
                yp = ps_a.tile([128, 512], FDT, tag="proj", name="yproj")
                for cb in range(8):
                    for c in range(2):
                        j = 2 * cb + c
                        mm(
                            yp[:, 0:256],
                            oh[c][:, hl * 1024 + cb * 128:
                                  hl * 1024 + cb * 128 + 128],
                            wot_sb[:, j * 256:(j + 1) * 256],
                            start=(j == 0), stop=(j == 15),
                        )
                ys = wpool.tile([128, 256], FDT, tag="ys", name="ys")
                nc.vector.tensor_copy(out=ys[:, :], in_=yp[:, 0:256])
                nc.sync.dma_start(
                    out=y[hl * 128:(hl + 1) * 128, :], in_=ys[:, :]
                )
    nc.finalize()
    return nc


def _prep_inputs(x, w_attn, b_attn, w_out):
    # causal mask on pseudo-positions s', in permuted order u = cb*128 + r
    rk = np.arange(128)
    rq = np.arange(512)
    masks = np.empty((16, 128, 512), dtype=ml_dtypes.bfloat16)
    for ki in range(8):
        for qj in range(2):
            spk = rk * 8 + ki                                # s' of key rows
            spq = (rq % 128) * 8 + (qj * 4 + rq // 128)      # s' of query cols
            masks[ki * 2 + qj] = np.where(
                spk[:, None] <= spq[None, :], 0.0, NEG
            ).astype(ml_dtypes.bfloat16)

    wqk = np.ascontiguousarray(
        np.concatenate([w_attn[0:2048] / 16.0, w_attn[2048:4096]]).T
    )  # (256, 4096)
    wvt = np.ascontiguousarray(w_attn[4096:6144].T)  # (256, 2048)
    bqk_arr = np.ascontiguousarray(
        np.concatenate([b_attn[0:2048] / 16.0, b_attn[2048:4096]])
        .reshape(32, 128).T
    )  # (128, 32)
    bv_arr = b_attn[4096:6144].reshape(1, 2048).astype(np.float32)
    wot_arr = np.ascontiguousarray(
        w_out.T.reshape(16, 128, 256).transpose(1, 0, 2).reshape(128, 4096)
    )

    in_maps = []
    for c in range(NCORES):
        b, g = divmod(c, 2)
        xt = np.ascontiguousarray(x[b, 512 * g:512 * (g + 1)].T)  # (256, 512)
        in_maps.append({
            "xt0": np.ascontiguousarray(xt[:128]),
            "xt1": np.ascontiguousarray(xt[128:]),
            "wqk0": np.ascontiguousarray(wqk[:128]),
            "wqk1": np.ascontiguousarray(wqk[128:]),
            "wv0": np.ascontiguousarray(wvt[:128]),
            "wv1": np.ascontiguousarray(wvt[128:]),
            "bqk": bqk_arr.astype(np.float32),
            "bv": bv_arr,
            "mask": masks,
            "wot": wot_arr.astype(np.float32),
            "onr": np.ones((1, 128), np.float32),
            "idn": np.eye(128, dtype=ml_dtypes.bfloat16),
            "onc": np.ones((128, 128), np.float32),
        })
    return in_maps


def kernel(x, w_attn, b_attn, w_out, b_out):
    x = np.asarray(x, dtype=np.float32)
    w_attn = np.asarray(w_attn, dtype=np.float32)
    b_attn = np.asarray(b_attn, dtype=np.float32)
    w_out = np.asarray(w_out, dtype=np.float32)
    b_out = np.asarray(b_out, dtype=np.float32)

    if "nc" not in _cache:
        _cache["nc"] = _build()
    nc = _cache["nc"]

    in_maps = _prep_inputs(x, w_attn, b_attn, w_out)
    res = run_bass_kernel_spmd(nc, in_maps, list(range(NCORES))).results

    out = np.empty((BS, SQL, EDIM), dtype=np.float32)
    for c in range(NCORES):
        b, g = divmod(c, 2)
        out[b, 512 * g:512 * (g + 1)] = res[c]["y"]
    out += b_out
    return out



# revision 3
# speedup vs baseline: 1.2961x; 1.2961x over previous
"""Masked multi-head attention on 8 NeuronCores (faithful torch raw-view semantics).

The reference reshapes (bs, sql, nh*edim) -> (bs, nh, sql, edim) as a RAW VIEW:
head h's length-1024 pseudo-sequence is built from x rows 128h..128h+127 (each
row contributes 8 pseudo-positions, one per 256-col block of the projection),
and output rows 128h..128h+128 depend only on head h. So the work splits into
32 independent (batch, head) pairs -> 4 per core, no cross-core reduction.

v2: NATURAL pseudo-position ordering (column u = s' = r*8 + cb, a stride-8
scatter at projection writeback) makes the causal mask block-triangular, so
only 36 of 64 score/PV 128x128 blocks per head are computed (the baseline's
permuted ordering made every block half-masked -> full 64). All attention
matmuls run in bf16 (1 cycle/row at 128-wide tiles). The in-block causal
triangle on diagonal blocks is injected INTO PSUM by one small matmul
(step-matrix @ shifted-NEG-diag) instead of a DVE mask add. exp runs on the
Act engine straight from PSUM in per-kb strips; softmax denominators come from
bf16 ones-matmuls accumulated per q-block; V is re-laid out k-major via PE
transposes. Q weights/bias pre-scaled by 1/16.
"""

import sys

sys.path.insert(0, "/opt/trn_rl_repo")

import ml_dtypes
import numpy as np

from concourse import bacc, mybir
from concourse.tile import TileContext
from concourse.bass_utils import run_bass_kernel_spmd

EDIM = 256
BS = 4
SQL = 1024
HPC = 4           # heads per core
NCORES = 8
FDT = mybir.dt.float32
BF = mybir.dt.bfloat16
NEG = -1.0e30

# strip kb covers q-blocks kb..7; OFF[kb] = col offset of strip kb in pt
OFF = [0]
for _kb in range(1, 8):
    OFF.append(OFF[-1] + (8 - _kb + 1) * 128)
# chunks of <=4 q-blocks per strip (PSUM bank = 512 fp32 cols)
CHUNKS = {kb: [list(range(kb, 8))[i:i + 4]
               for i in range(0, 8 - kb, 4)] for kb in range(8)}

_cache = {}


def _build():
    nc = bacc.Bacc(dynamic_dma_scratch_size=512)

    xt0 = nc.declare_dram_parameter("xt0", [128, 512], BF, isOutput=False)
    xt1 = nc.declare_dram_parameter("xt1", [128, 512], BF, isOutput=False)
    wqk0 = nc.declare_dram_parameter("wqk0", [128, 4096], BF, isOutput=False)
    wqk1 = nc.declare_dram_parameter("wqk1", [128, 4096], BF, isOutput=False)
    wv0 = nc.declare_dram_parameter("wv0", [128, 2048], BF, isOutput=False)
    wv1 = nc.declare_dram_parameter("wv1", [128, 2048], BF, isOutput=False)
    bqk = nc.declare_dram_parameter("bqk", [128, 32], FDT, isOutput=False)
    bvp = nc.declare_dram_parameter("bvp", [128, 16], FDT, isOutput=False)
    wot = nc.declare_dram_parameter("wot", [128, 4096], BF, isOutput=False)
    stepA = nc.declare_dram_parameter("stepA", [128, 128], BF, isOutput=False)
    negB = nc.declare_dram_parameter("negB", [128, 128], BF, isOutput=False)
    onesc = nc.declare_dram_parameter("onesc", [128, 128], BF, isOutput=False)
    idn = nc.declare_dram_parameter("idn", [128, 128], BF, isOutput=False)
    y = nc.declare_dram_parameter("y", [512, 256], FDT, isOutput=True)

    EXP = mybir.ActivationFunctionType.Exp
    IDF = mybir.ActivationFunctionType.Identity

    with TileContext(nc) as tc:
        with (
            tc.tile_pool(name="const", bufs=1) as cpool,
            tc.tile_pool(name="big", bufs=1) as bpool,
            tc.tile_pool(name="pt", bufs=2) as ptpool,
            tc.tile_pool(name="oh", bufs=4) as ohpool,
            tc.tile_pool(name="rc", bufs=4) as rcpool,
            tc.tile_pool(name="ys", bufs=2) as yspool,
            tc.tile_pool(name="ps_a", bufs=3, space="PSUM") as poolA,
        ):
            def load(pool, name, src, shape, dt=FDT, tag=None):
                t = pool.tile(shape, dt, tag=tag or name, name=name)
                nc.sync.dma_start(out=t[:, :], in_=src)
                return t

            def mm(out, lhsT, rhs, **kw):
                nc.tensor.matmul(out, lhsT, rhs, **kw)

            # ---- DMAs in consumption order ----
            xt_sb = [load(cpool, "xt0", xt0[:, :], [128, 512], dt=BF),
                     load(cpool, "xt1", xt1[:, :], [128, 512], dt=BF)]
            bqk_sb = load(cpool, "bqk", bqk[:, :], [128, 32])
            stepA_sb = load(cpool, "stepA", stepA[:, :], [128, 128], dt=BF)
            negB_sb = load(cpool, "negB", negB[:, :], [128, 128], dt=BF)
            ones_sb = load(cpool, "onesc", onesc[:, :], [128, 128], dt=BF)
            idn_sb = load(cpool, "idn", idn[:, :], [128, 128], dt=BF)
            bvp_sb = load(cpool, "bvp", bvp[:, :], [128, 16])

            wqk_srcs = [wqk0, wqk1]
            wqk_sb = []
            for d in range(2):
                wqk_sb.append(bpool.tile([128, 4096], BF, tag=f"wqk{d}",
                                         name=f"wqk{d}"))
            for s in range(2):          # Q cols then K cols
                for d in range(2):
                    for ch in range(2):
                        c0 = s * 2048 + ch * 1024
                        nc.sync.dma_start(
                            out=wqk_sb[d][:, c0:c0 + 1024],
                            in_=wqk_srcs[d][:, c0:c0 + 1024],
                        )
            wv_srcs = [wv0, wv1]
            wv_sb = []
            for d in range(2):
                t = bpool.tile([128, 2048], BF, tag=f"wv{d}", name=f"wv{d}")
                for ch in range(2):
                    nc.sync.dma_start(
                        out=t[:, ch * 1024:(ch + 1) * 1024],
                        in_=wv_srcs[d][:, ch * 1024:(ch + 1) * 1024],
                    )
                wv_sb.append(t)
            wot_sb = bpool.tile([128, 4096], BF, tag="wot", name="wot")
            for ch in range(2):
                nc.sync.dma_start(
                    out=wot_sb[:, ch * 2048:(ch + 1) * 2048],
                    in_=wot[:, ch * 2048:(ch + 1) * 2048],
                )

            # d-major Q^T/K^T/V^T, NATURAL order: cols = hl*1024 + s'
            qt = [bpool.tile([128, 4096], BF, tag=f"qt{c}", name=f"qt{c}")
                  for c in range(2)]
            kt = [bpool.tile([128, 4096], BF, tag=f"kt{c}", name=f"kt{c}")
                  for c in range(2)]
            vt = [bpool.tile([128, 4096], BF, tag=f"vt{c}", name=f"vt{c}")
                  for c in range(2)]
            # V k-major per head: [128 k, kb*256 + c*128 + d]
            v_nat = [bpool.tile([128, 2048], BF, tag=f"vn{hl}", name=f"vn{hl}")
                     for hl in range(HPC)]

            # ---- P1: Q^T/K^T projections (all heads), natural scatter ----
            nbias = 0
            for s in range(2):
                dst = qt if s == 0 else kt
                for cb in range(8):
                    for c in range(2):
                        ps = poolA.tile([128, 512], FDT, tag="pa", name="proj")
                        for d in range(2):
                            mm(
                                ps[:, :],
                                wqk_sb[d][:, s * 2048 + cb * 256 + c * 128:
                                          s * 2048 + cb * 256 + c * 128 + 128],
                                xt_sb[d][:, :],
                                start=(d == 0), stop=(d == 1),
                            )
                        bi = s * 16 + cb * 2 + c
                        out_v = dst[c].rearrange(
                            "p (h r e) -> p h r e", h=4, r=128, e=8
                        )[:, :, :, cb]
                        in_v = ps.rearrange("p (h r) -> p h r", h=4)[:, :, :]
                        if nbias % 2 == 0:
                            nc.scalar.activation(
                                out_v, in_v, IDF, bias=bqk_sb[:, bi:bi + 1]
                            )
                        else:
                            nc.vector.tensor_scalar_add(
                                out=out_v, in0=in_v,
                                scalar1=bqk_sb[:, bi:bi + 1],
                            )
                        nbias += 1

            # ---- P2: V^T projection (all heads), natural scatter ----
            for cb in range(8):
                for c in range(2):
                    ps = poolA.tile([128, 512], FDT, tag="pa", name="vproj")
                    for d in range(2):
                        mm(
                            ps[:, :],
                            wv_sb[d][:, cb * 256 + c * 128:
                                     cb * 256 + c * 128 + 128],
                            xt_sb[d][:, :],
                            start=(d == 0), stop=(d == 1),
                        )
                    bi = cb * 2 + c
                    out_v = vt[c].rearrange(
                        "p (h r e) -> p h r e", h=4, r=128, e=8
                    )[:, :, :, cb]
                    in_v = ps.rearrange("p (h r) -> p h r", h=4)[:, :, :]
                    if nbias % 2 == 0:
                        nc.scalar.activation(
                            out_v, in_v, IDF, bias=bvp_sb[:, bi:bi + 1]
                        )
                    else:
                        nc.vector.tensor_scalar_add(
                            out=out_v, in0=in_v, scalar1=bvp_sb[:, bi:bi + 1]
                        )
                    nbias += 1

            # ---- P2b: V -> k-major via PE transposes ----
            with tc.tile_pool(name="ps_t", bufs=2, space="PSUM") as tpool:
                ncp = 0
                for hl in range(HPC):
                    for kb in range(8):
                        tp = tpool.tile([128, 256], BF, tag="tp", name="tp")
                        for c in range(2):
                            nc.tensor.transpose(
                                tp[:, c * 128:(c + 1) * 128],
                                vt[c][:, hl * 1024 + kb * 128:
                                      hl * 1024 + (kb + 1) * 128],
                                idn_sb[:, :],
                            )
                        dst = v_nat[hl][:, kb * 256:(kb + 1) * 256]
                        if ncp % 2 == 0:
                            nc.scalar.copy(dst, tp[:, :])
                        else:
                            nc.vector.tensor_copy(out=dst, in_=tp[:, :])
                        ncp += 1

            # ---- P3: attention per head, causal block-skipped ----
            with (
                tc.tile_pool(name="ps_se", bufs=2, space="PSUM") as pse,
                tc.tile_pool(name="ps_po", bufs=2, space="PSUM") as ppo,
            ):
                for hl in range(HPC):
                    qoff = hl * 1024
                    pt_h = ptpool.tile([128, 4608], BF, tag="pt", name="pt")
                    # scores + exp, strip-major over kb
                    for kb in range(8):
                        for chunk in CHUNKS[kb]:
                            n = len(chunk) * 128
                            sp = poolA.tile([128, 512], FDT, tag="pa",
                                            name="score")
                            for i, qb in enumerate(chunk):
                                r0 = i * 128
                                strt = True
                                if qb == kb:
                                    mm(sp[:, r0:r0 + 128], stepA_sb[:, :],
                                       negB_sb[:, :], start=True, stop=False)
                                    strt = False
                                for c in range(2):
                                    mm(
                                        sp[:, r0:r0 + 128],
                                        kt[c][:, qoff + kb * 128:
                                              qoff + kb * 128 + 128],
                                        qt[c][:, qoff + qb * 128:
                                              qoff + qb * 128 + 128],
                                        start=(strt and c == 0), stop=(c == 1),
                                    )
                            o0 = OFF[kb] + (chunk[0] - kb) * 128
                            nc.scalar.activation(
                                pt_h[:, o0:o0 + n], sp[:, 0:n], EXP
                            )
                    # denominators + PV + normalize, q-block major
                    oh = [ohpool.tile([128, 1024], BF, tag="oh",
                                      name=f"oh{c}") for c in range(2)]
                    for qb in range(8):
                        se = pse.tile([128, 128], FDT, tag="se", name="se")
                        po = ppo.tile([128, 256], FDT, tag="po", name="po")

                        def ptr(kb, qb=qb):
                            o = OFF[kb] + (qb - kb) * 128
                            return pt_h[:, o:o + 128]

                        for kb in range(qb + 1):
                            mm(se[:, :], ones_sb[:, :], ptr(kb),
                               start=(kb == 0), stop=(kb == qb))
                        # one full accumulation chain per c: a start=True marks
                        # the WHOLE psum bank pending-zero, so chains in the
                        # same bank must not interleave mid-accumulation
                        for c in range(2):
                            for kb in range(qb + 1):
                                mm(
                                    po[:, c * 128:(c + 1) * 128],
                                    v_nat[hl][:, kb * 256 + c * 128:
                                              kb * 256 + c * 128 + 128],
                                    ptr(kb),
                                    start=(kb == 0), stop=(kb == qb),
                                )
                        rc = rcpool.tile([128, 128], FDT, tag="rc", name="rc")
                        nc.vector.reciprocal(out=rc[:, :], in_=se[:, :])
                        for c in range(2):
                            nc.vector.tensor_mul(
                                out=oh[c][:, qb * 128:(qb + 1) * 128],
                                in0=po[:, c * 128:(c + 1) * 128],
                                in1=rc[:, :],
                            )

                    # ---- P4: output projection for this head ----
                    yp = ppo.tile([128, 256], FDT, tag="po", name="yproj")
                    for cb in range(8):
                        for c in range(2):
                            j = cb * 2 + c
                            mm(
                                yp[:, 0:256],
                                oh[c].rearrange(
                                    "p (r e) -> p r e", r=128, e=8
                                )[:, :, cb],
                                wot_sb[:, j * 256:(j + 1) * 256],
                                start=(j == 0), stop=(j == 15),
                            )
                    ys = yspool.tile([128, 256], FDT, tag="ys", name="ys")
                    nc.scalar.copy(ys[:, :], yp[:, 0:256])
                    nc.sync.dma_start(
                        out=y[hl * 128:(hl + 1) * 128, :], in_=ys[:, :]
                    )
    nc.finalize()
    return nc


def _prep_inputs(x, w_attn, b_attn, w_out):
    b16 = ml_dtypes.bfloat16
    wqk = np.ascontiguousarray(
        np.concatenate([w_attn[0:2048] / 16.0, w_attn[2048:4096]]).T
    ).astype(b16)  # (256, 4096)
    wvt = np.ascontiguousarray(w_attn[4096:6144].T).astype(b16)  # (256, 2048)
    bqk_arr = np.ascontiguousarray(
        np.concatenate([b_attn[0:2048] / 16.0, b_attn[2048:4096]])
        .reshape(32, 128).T
    ).astype(np.float32)  # (128, 32)
    bvp_arr = np.ascontiguousarray(
        b_attn[4096:6144].reshape(16, 128).T
    ).astype(np.float32)  # (128, 16)
    wot_arr = np.ascontiguousarray(
        w_out.T.reshape(16, 128, 256).transpose(1, 0, 2).reshape(128, 4096)
    ).astype(b16)

    stepA = np.triu(np.ones((128, 128), np.float32)).astype(b16)
    negB = (NEG * np.eye(128, k=-1)).astype(b16)
    onesc = np.ones((128, 128), np.float32).astype(b16)
    idn = np.eye(128, dtype=np.float32).astype(b16)

    in_maps = []
    for cidx in range(NCORES):
        b, g = divmod(cidx, 2)
        xt = np.ascontiguousarray(
            x[b, 512 * g:512 * (g + 1)].T
        ).astype(b16)  # (256, 512)
        in_maps.append({
            "xt0": np.ascontiguousarray(xt[:128]),
            "xt1": np.ascontiguousarray(xt[128:]),
            "wqk0": np.ascontiguousarray(wqk[:128]),
            "wqk1": np.ascontiguousarray(wqk[128:]),
            "wv0": np.ascontiguousarray(wvt[:128]),
            "wv1": np.ascontiguousarray(wvt[128:]),
            "bqk": bqk_arr,
            "bvp": bvp_arr,
            "wot": wot_arr,
            "stepA": stepA,
            "negB": negB,
            "onesc": onesc,
            "idn": idn,
        })
    return in_maps


def kernel(x, w_attn, b_attn, w_out, b_out):
    x = np.asarray(x, dtype=np.float32)
    w_attn = np.asarray(w_attn, dtype=np.float32)
    b_attn = np.asarray(b_attn, dtype=np.float32)
    w_out = np.asarray(w_out, dtype=np.float32)
    b_out = np.asarray(b_out, dtype=np.float32)

    if "nc" not in _cache:
        _cache["nc"] = _build()
    nc = _cache["nc"]

    in_maps = _prep_inputs(x, w_attn, b_attn, w_out)
    res = run_bass_kernel_spmd(nc, in_maps, list(range(NCORES))).results

    out = np.empty((BS, SQL, EDIM), dtype=np.float32)
    for c in range(NCORES):
        b, g = divmod(c, 2)
        out[b, 512 * g:512 * (g + 1)] = res[c]["y"]
    out += b_out
    return out
